# revision 3
# baseline (speedup 1.0000x reference)
"""MoE routing kernel for Trainium2 (8 NeuronCores).

Problem: out[b,l,:] = actions[b,l,:]              if action_type[b,l] == 0
         out[b,l,:] = W[t-1] @ actions[b,l,:] + b[t-1]   if action_type == t >= 1

Strategy (type-parallel): instead of computing all 7 expert projections for
every token (the dense reference), route each token to the single expert it
needs. The host groups the B*L tokens by action_type; core t processes the
tokens of type t as one dense [C, D] x [D, D] matmul (core 0 gets the
identity type with W = I so the SPMD program is uniform across cores; its
result is discarded and identity rows are copied exactly on the host).
Token data is packed transposed ([D, C], feature on partition axis) so the
device does pure dense streaming matmuls - no gather/transpose on device.

Per-core work: ~(B*L/8) tokens * 2*D^2 flops (fp32r ~= full PE rate) and
~4MB weights + 2*C*4KB activations of HBM traffic - right at the
compute/memory ridge.
"""

import sys

for _p in ("/root/.axon_site/_ro/trn_rl_repo", "/opt/trn_rl_repo"):
    if _p not in sys.path:
        sys.path.append(_p)

import numpy as np
import concourse.bass as bass
import concourse.tile as tile
from concourse import bacc, mybir
from concourse.bass_utils import run_bass_kernel_spmd

D = 1024
P = 128
N_CORES = 8
TT = 512  # token tile (moving-dim block)
F32 = mybir.dt.float32
F32R = mybir.dt.float32r

_program_cache: dict[int, bass.Bass] = {}


def build_program(C: int) -> bass.Bass:
    """out[C, D] = xT.T @ wT + bB   (xT: [D, C], wT: [D, D] both [contract, free])."""
    if C in _program_cache:
        return _program_cache[C]
    nc = bacc.Bacc("TRN2", target_bir_lowering=False, debug=False, num_devices=N_CORES)
    # Inputs declared fp32r so matmul operands come straight from DMA with no
    # on-device conversion pass (fp32r rounding happens in the PE datapath).
    xT = nc.dram_tensor("xT", [D, C], F32R, kind="ExternalInput")
    wT = nc.dram_tensor("wT", [D, D], F32R, kind="ExternalInput")
    bB = nc.dram_tensor("bB", [P, D], F32, kind="ExternalInput")
    out = nc.dram_tensor("out", [C, D], F32, kind="ExternalOutput")

    n_ic = D // P  # contraction chunks
    n_ob = D // TT  # output blocks

    # Token-tile schedule: small leading tiles so the first matmuls only wait
    # for ~2.5MB of DMA instead of the full 6MB weight+x preload.
    t_tiles = []
    t0 = 0
    for sz in (P, 3 * P):
        if t0 < C:
            tt = min(sz, C - t0)
            t_tiles.append((t0, tt))
            t0 += tt
    while t0 < C:
        tt = min(TT, C - t0)
        t_tiles.append((t0, tt))
        t0 += tt

    with tile.TileContext(nc) as tc:
        with (
            tc.tile_pool(name="wpool", bufs=1) as wpool,
            tc.tile_pool(name="bpool", bufs=1) as bpool,
            tc.tile_pool(name="xpool", bufs=3) as xpool,
            tc.tile_pool(name="opool", bufs=4) as opool,
            tc.tile_pool(name="psum", bufs=8, space="PSUM") as psum_pool,
        ):
            # First token tile's x chunks, then weight halves for ob=0 —
            # interleaved in consumption order of the first matmul group.
            first_t0, first_tt = t_tiles[0]
            x_first = []
            for ic in range(n_ic):
                xt = xpool.tile([P, first_tt], F32R, tag=f"x{ic}")
                nc.sync.dma_start(xt[:], xT[ic * P : (ic + 1) * P, :first_tt])
                x_first.append(xt)
            # Expert weights resident in SBUF, split into (ic, ob) halves.
            w_tiles = [[None] * n_ob for _ in range(n_ic)]
            for ob in range(n_ob):
                for ic in range(n_ic):
                    wt = wpool.tile([P, TT], F32R, tag=f"w{ic}_{ob}")
                    nc.sync.dma_start(
                        wt[:], wT[ic * P : (ic + 1) * P, ob * TT : (ob + 1) * TT]
                    )
                    w_tiles[ic][ob] = wt
            b_tile = bpool.tile([P, D], F32)
            nc.sync.dma_start(b_tile[:], bB[:])

            for ti, (t0, tt) in enumerate(t_tiles):
                if ti == 0:
                    x_tiles = x_first
                else:
                    x_tiles = []
                    for ic in range(n_ic):
                        xt = xpool.tile([P, tt], F32R, tag=f"x{ic}")
                        nc.sync.dma_start(
                            xt[:], xT[ic * P : (ic + 1) * P, t0 : t0 + tt]
                        )
                        x_tiles.append(xt)
                for tc_ in range(tt // P):  # 128-token chunks (psum partitions)
                    for ob in range(n_ob):  # 512-wide output blocks
                        ps = psum_pool.tile([P, TT], F32)
                        for ic in range(n_ic):
                            nc.tensor.matmul(
                                ps[:],
                                x_tiles[ic][:, tc_ * P : (tc_ + 1) * P],
                                w_tiles[ic][ob][:],
                                start=(ic == 0),
                                stop=(ic == n_ic - 1),
                            )
                        ot = opool.tile([P, TT], F32)
                        nc.vector.tensor_add(
                            ot[:], ps[:], b_tile[:, ob * TT : (ob + 1) * TT]
                        )
                        nc.sync.dma_start(
                            out[
                                t0 + tc_ * P : t0 + (tc_ + 1) * P,
                                ob * TT : (ob + 1) * TT,
                            ],
                            ot[:],
                        )
    nc.compile()
    _program_cache[C] = nc
    return nc


def kernel(actions, action_type, W, b, _trace=False):
    actions = np.ascontiguousarray(actions, dtype=np.float32)
    B, L, _ = actions.shape
    flat = actions.reshape(B * L, D)
    types = np.asarray(action_type).reshape(B * L).astype(np.int64)

    idx = [np.flatnonzero(types == t) for t in range(N_CORES)]
    counts = [len(i) for i in idx]
    C = max(P, -(-max(counts) // P) * P)  # pad to multiple of 128

    W = np.asarray(W, dtype=np.float32)
    b_np = np.asarray(b, dtype=np.float32)
    eye = np.eye(D, dtype=np.float32)

    in_maps = []
    for t in range(N_CORES):
        xT = np.zeros((D, C), dtype=np.float32)
        if t > 0 and counts[t]:
            xT[:, : counts[t]] = flat[idx[t]].T
        wT = eye if t == 0 else np.ascontiguousarray(W[t - 1].T)
        bvec = np.zeros(D, dtype=np.float32) if t == 0 else b_np[t - 1]
        bB = np.ascontiguousarray(np.broadcast_to(bvec, (P, D)))
        in_maps.append({"xT": xT, "wT": wT, "bB": bB})

    nc = build_program(C)
    r = run_bass_kernel_spmd(nc, in_maps, list(range(N_CORES)), trace=_trace)

    out_flat = np.empty_like(flat)
    out_flat[idx[0]] = flat[idx[0]]  # identity tokens: exact copy
    for t in range(1, N_CORES):
        if counts[t]:
            out_flat[idx[t]] = r.results[t]["out"][: counts[t]]
    out = out_flat.reshape(B, L, D)
    if _trace:
        return out, r
    return out


# revision 7
# speedup vs baseline: 1.0699x; 1.0699x over previous
"""MoE routing kernel for Trainium2 (8 NeuronCores).

Problem: out[b,l,:] = actions[b,l,:]              if action_type[b,l] == 0
         out[b,l,:] = W[t-1] @ actions[b,l,:] + b[t-1]   if action_type == t >= 1

Strategy (type-parallel): instead of computing all 7 expert projections for
every token (the dense reference), route each token to the single expert it
needs. The host groups the B*L tokens by action_type; core t processes the
tokens of type t as one dense [C, D] x [D, D] matmul (core 0 gets the
identity type with W = I so the SPMD program is uniform across cores; its
result is discarded and identity rows are copied exactly on the host).
Token data is packed transposed ([D, C], feature on partition axis) so the
device does pure dense streaming matmuls - no gather/transpose on device.

Per-core work: ~(B*L/8) tokens * 2*D^2 flops (fp32r ~= full PE rate) and
~4MB weights + 2*C*4KB activations of HBM traffic - right at the
compute/memory ridge.
"""

import sys

for _p in ("/root/.axon_site/_ro/trn_rl_repo", "/opt/trn_rl_repo"):
    if _p not in sys.path:
        sys.path.append(_p)

import numpy as np
import concourse.bass as bass
import concourse.tile as tile
from concourse import bacc, mybir
from concourse.bass_utils import run_bass_kernel_spmd

D = 1024
P = 128
N_CORES = 8
TT = 512  # token tile (moving-dim block)
F32 = mybir.dt.float32
F32R = mybir.dt.float32r

_program_cache: dict[int, bass.Bass] = {}


def build_program(C: int) -> bass.Bass:
    """out[C, D] = xT.T @ wT + bB   (xT: [D, C], wT: [D, D] both [contract, free])."""
    if C in _program_cache:
        return _program_cache[C]
    nc = bacc.Bacc("TRN2", target_bir_lowering=False, debug=False, num_devices=N_CORES)
    # Inputs declared fp32r so matmul operands come straight from DMA with no
    # on-device conversion pass (fp32r rounding happens in the PE datapath).
    xT = nc.dram_tensor("xT", [D, C], F32R, kind="ExternalInput")
    wT = nc.dram_tensor("wT", [D, D], F32R, kind="ExternalInput")
    bB = nc.dram_tensor("bB", [P, D], F32, kind="ExternalInput")
    out = nc.dram_tensor("out", [C, D], F32, kind="ExternalOutput")

    n_ic = D // P  # contraction chunks
    n_ob = D // TT  # output blocks

    # Uniform 512-token tiles (last = remainder).
    t_tiles = []
    t0 = 0
    while t0 < C:
        tt = min(TT, C - t0)
        t_tiles.append((t0, tt))
        t0 += tt

    with tile.TileContext(nc) as tc:
        with (
            tc.tile_pool(name="wpool", bufs=1) as wpool,
            tc.tile_pool(name="bpool", bufs=1) as bpool,
            tc.tile_pool(name="xpool", bufs=3) as xpool,
            tc.tile_pool(name="opool", bufs=4) as opool,
            tc.tile_pool(name="psum", bufs=1, space="PSUM") as psum_pool,
        ):
            # First token tile's x chunks go first so the first matmul only
            # waits for x(t0) + w[0]; then weights in ic order (both output
            # halves per ic) so the ic-outer matmul schedule consumes each
            # chunk just as it lands.
            first_tt = t_tiles[0][1]
            x_first = []
            for ic in range(n_ic):
                xt = xpool.tile([P, first_tt], F32R, tag=f"x{ic}")
                nc.sync.dma_start(xt[:], xT[ic * P : (ic + 1) * P, :first_tt])
                x_first.append(xt)
            w_tiles = [[None] * n_ob for _ in range(n_ic)]
            for ic in range(n_ic):
                for ob in range(n_ob):
                    wt = wpool.tile([P, TT], F32R, tag=f"w{ic}_{ob}")
                    nc.sync.dma_start(
                        wt[:], wT[ic * P : (ic + 1) * P, ob * TT : (ob + 1) * TT]
                    )
                    w_tiles[ic][ob] = wt
            b_tile = bpool.tile([P, D], F32)
            nc.sync.dma_start(b_tile[:], bB[:])

            for ti, (t0, tt) in enumerate(t_tiles):
                if ti == 0:
                    x_tiles = x_first
                else:
                    x_tiles = []
                    for ic in range(n_ic):
                        xt = xpool.tile([P, tt], F32R, tag=f"x{ic}")
                        nc.sync.dma_start(
                            xt[:], xT[ic * P : (ic + 1) * P, t0 : t0 + tt]
                        )
                        x_tiles.append(xt)
                # One psum bank per (token-chunk, output-block) group; run the
                # contraction ic-outer across all banks so each weight chunk
                # is needed only once per ~8 matmuls (JIT weight streaming).
                groups = [
                    (tc_, ob) for tc_ in range(tt // P) for ob in range(n_ob)
                ]
                ps = {
                    g: psum_pool.tile(
                        [P, TT], F32, name=f"ps_{ti}_{gi}", tag=f"ps{gi % 8}"
                    )
                    for gi, g in enumerate(groups)
                }
                for ic in range(n_ic):
                    for tc_, ob in groups:
                        nc.tensor.matmul(
                            ps[(tc_, ob)][:],
                            x_tiles[ic][:, tc_ * P : (tc_ + 1) * P],
                            w_tiles[ic][ob][:],
                            start=(ic == 0),
                            stop=(ic == n_ic - 1),
                        )
                for tc_, ob in groups:
                    ot = opool.tile([P, TT], F32)
                    nc.vector.tensor_add(
                        ot[:], ps[(tc_, ob)][:], b_tile[:, ob * TT : (ob + 1) * TT]
                    )
                    nc.sync.dma_start(
                        out[
                            t0 + tc_ * P : t0 + (tc_ + 1) * P,
                            ob * TT : (ob + 1) * TT,
                        ],
                        ot[:],
                    )
    nc.compile()
    _program_cache[C] = nc
    return nc


def kernel(actions, action_type, W, b, _trace=False):
    actions = np.ascontiguousarray(actions, dtype=np.float32)
    B, L, _ = actions.shape
    flat = actions.reshape(B * L, D)
    types = np.asarray(action_type).reshape(B * L).astype(np.int64)

    idx = [np.flatnonzero(types == t) for t in range(N_CORES)]
    counts = [len(i) for i in idx]
    C = max(P, -(-max(counts) // P) * P)  # pad to multiple of 128

    W = np.asarray(W, dtype=np.float32)
    b_np = np.asarray(b, dtype=np.float32)
    eye = np.eye(D, dtype=np.float32)

    in_maps = []
    for t in range(N_CORES):
        xT = np.zeros((D, C), dtype=np.float32)
        if t > 0 and counts[t]:
            xT[:, : counts[t]] = flat[idx[t]].T
        wT = eye if t == 0 else np.ascontiguousarray(W[t - 1].T)
        bvec = np.zeros(D, dtype=np.float32) if t == 0 else b_np[t - 1]
        bB = np.ascontiguousarray(np.broadcast_to(bvec, (P, D)))
        in_maps.append({"xT": xT, "wT": wT, "bB": bB})

    nc = build_program(C)
    r = run_bass_kernel_spmd(nc, in_maps, list(range(N_CORES)), trace=_trace)

    out_flat = np.empty_like(flat)
    out_flat[idx[0]] = flat[idx[0]]  # identity tokens: exact copy
    for t in range(1, N_CORES):
        if counts[t]:
            out_flat[idx[t]] = r.results[t]["out"][: counts[t]]
    out = out_flat.reshape(B, L, D)
    if _trace:
        return out, r
    return out


# revision 9
# speedup vs baseline: 1.0744x; 1.0042x over previous
"""MoE routing kernel for Trainium2 (8 NeuronCores).

Problem: out[b,l,:] = actions[b,l,:]              if action_type[b,l] == 0
         out[b,l,:] = W[t-1] @ actions[b,l,:] + b[t-1]   if action_type == t >= 1

Strategy (type-parallel): instead of computing all 7 expert projections for
every token (the dense reference), route each token to the single expert it
needs. The host groups the B*L tokens by action_type; core t processes the
tokens of type t as one dense [C, D] x [D, D] matmul (core 0 gets the
identity type with W = I so the SPMD program is uniform across cores; its
result is discarded and identity rows are copied exactly on the host).
Token data is packed transposed ([D, C], feature on partition axis) so the
device does pure dense streaming matmuls - no gather/transpose on device.

Per-core work: ~(B*L/8) tokens * 2*D^2 flops (fp32r ~= full PE rate) and
~4MB weights + 2*C*4KB activations of HBM traffic - right at the
compute/memory ridge.
"""

import sys

for _p in ("/root/.axon_site/_ro/trn_rl_repo", "/opt/trn_rl_repo"):
    if _p not in sys.path:
        sys.path.append(_p)

import numpy as np
import concourse.bass as bass
import concourse.tile as tile
from concourse import bacc, mybir
from concourse.bass_utils import run_bass_kernel_spmd

D = 1024
P = 128
N_CORES = 8
TT = 512  # token tile (moving-dim block)
F32 = mybir.dt.float32
F32R = mybir.dt.float32r

_program_cache: dict[int, bass.Bass] = {}


def build_program(C: int) -> bass.Bass:
    """out[C, D] = xT.T @ wT + bB   (xT: [D, C], wT: [D, D] both [contract, free])."""
    if C in _program_cache:
        return _program_cache[C]
    nc = bacc.Bacc("TRN2", target_bir_lowering=False, debug=False, num_devices=N_CORES)
    # Inputs declared fp32r so matmul operands come straight from DMA with no
    # on-device conversion pass (fp32r rounding happens in the PE datapath).
    xT = nc.dram_tensor("xT", [D, C], F32R, kind="ExternalInput")
    wT = nc.dram_tensor("wT", [D, D], F32R, kind="ExternalInput")
    bB = nc.dram_tensor("bB", [P, D], F32, kind="ExternalInput")
    out = nc.dram_tensor("out", [C, D], F32, kind="ExternalOutput")

    n_ic = D // P  # contraction chunks
    n_ob = D // TT  # output blocks

    # Uniform 512-token tiles (last = remainder).
    t_tiles = []
    t0 = 0
    while t0 < C:
        tt = min(TT, C - t0)
        t_tiles.append((t0, tt))
        t0 += tt

    with tile.TileContext(nc) as tc:
        with (
            tc.tile_pool(name="wpool", bufs=1) as wpool,
            tc.tile_pool(name="bpool", bufs=1) as bpool,
            tc.tile_pool(name="xpool", bufs=3) as xpool,
            tc.tile_pool(name="opool", bufs=4) as opool,
            tc.tile_pool(name="psum", bufs=1, space="PSUM") as psum_pool,
        ):
            # Interleave x(t0)[ic] with w[ic] in ic order: the ic-outer matmul
            # schedule consumes exactly (x[ic], w[ic]) per stage, so the PE
            # starts after ~1MB of DMA and self-paces against the stream.
            first_tt = t_tiles[0][1]
            x_first = []
            w_tiles = []
            b_tile = None
            for ic in range(n_ic):
                xt = xpool.tile([P, first_tt], F32R, name=f"x_first{ic}", tag=f"x{ic}")
                nc.sync.dma_start(xt[:], xT[ic * P : (ic + 1) * P, :first_tt])
                x_first.append(xt)
                wt = wpool.tile([P, D], F32R, name=f"w{ic}", tag=f"w{ic}")
                nc.sync.dma_start(wt[:], wT[ic * P : (ic + 1) * P, :])
                w_tiles.append(wt)
                if ic == 2:
                    b_tile = bpool.tile([P, D], F32, name="b_tile")
                    nc.sync.dma_start(b_tile[:], bB[:])

            for ti, (t0, tt) in enumerate(t_tiles):
                if ti == 0:
                    x_tiles = x_first
                else:
                    x_tiles = []
                    for ic in range(n_ic):
                        xt = xpool.tile([P, tt], F32R, tag=f"x{ic}")
                        nc.sync.dma_start(
                            xt[:], xT[ic * P : (ic + 1) * P, t0 : t0 + tt]
                        )
                        x_tiles.append(xt)
                # One psum bank per (token-chunk, output-block) group; run the
                # contraction ic-outer across all banks so each weight chunk
                # is needed only once per ~8 matmuls (JIT weight streaming).
                groups = [
                    (tc_, ob) for tc_ in range(tt // P) for ob in range(n_ob)
                ]
                ps = {
                    g: psum_pool.tile(
                        [P, TT], F32, name=f"ps_{ti}_{gi}", tag=f"ps{gi % 8}"
                    )
                    for gi, g in enumerate(groups)
                }
                for ic in range(n_ic):
                    for tc_, ob in groups:
                        nc.tensor.matmul(
                            ps[(tc_, ob)][:],
                            x_tiles[ic][:, tc_ * P : (tc_ + 1) * P],
                            w_tiles[ic][:, ob * TT : (ob + 1) * TT],
                            start=(ic == 0),
                            stop=(ic == n_ic - 1),
                        )
                for tc_, ob in groups:
                    ot = opool.tile([P, TT], F32)
                    nc.vector.tensor_add(
                        ot[:], ps[(tc_, ob)][:], b_tile[:, ob * TT : (ob + 1) * TT]
                    )
                    nc.sync.dma_start(
                        out[
                            t0 + tc_ * P : t0 + (tc_ + 1) * P,
                            ob * TT : (ob + 1) * TT,
                        ],
                        ot[:],
                    )
    nc.compile()
    _program_cache[C] = nc
    return nc


def kernel(actions, action_type, W, b, _trace=False):
    actions = np.ascontiguousarray(actions, dtype=np.float32)
    B, L, _ = actions.shape
    flat = actions.reshape(B * L, D)
    types = np.asarray(action_type).reshape(B * L).astype(np.int64)

    idx = [np.flatnonzero(types == t) for t in range(N_CORES)]
    counts = [len(i) for i in idx]
    C = max(P, -(-max(counts) // P) * P)  # pad to multiple of 128

    W = np.asarray(W, dtype=np.float32)
    b_np = np.asarray(b, dtype=np.float32)
    eye = np.eye(D, dtype=np.float32)

    in_maps = []
    for t in range(N_CORES):
        xT = np.zeros((D, C), dtype=np.float32)
        if t > 0 and counts[t]:
            xT[:, : counts[t]] = flat[idx[t]].T
        wT = eye if t == 0 else np.ascontiguousarray(W[t - 1].T)
        bvec = np.zeros(D, dtype=np.float32) if t == 0 else b_np[t - 1]
        bB = np.ascontiguousarray(np.broadcast_to(bvec, (P, D)))
        in_maps.append({"xT": xT, "wT": wT, "bB": bB})

    nc = build_program(C)
    r = run_bass_kernel_spmd(nc, in_maps, list(range(N_CORES)), trace=_trace)

    out_flat = np.empty_like(flat)
    out_flat[idx[0]] = flat[idx[0]]  # identity tokens: exact copy
    for t in range(1, N_CORES):
        if counts[t]:
            out_flat[idx[t]] = r.results[t]["out"][: counts[t]]
    out = out_flat.reshape(B, L, D)
    if _trace:
        return out, r
    return out


# revision 10
# speedup vs baseline: 1.1561x; 1.0760x over previous
"""MoE routing kernel for Trainium2 (8 NeuronCores).

Problem: out[b,l,:] = actions[b,l,:]              if action_type[b,l] == 0
         out[b,l,:] = W[t-1] @ actions[b,l,:] + b[t-1]   if action_type == t >= 1

Strategy (type-parallel): instead of computing all 7 expert projections for
every token (the dense reference), route each token to the single expert it
needs. The host groups the B*L tokens by action_type; core t processes the
tokens of type t as one dense [C, D] x [D, D] matmul (core 0 gets the
identity type with W = I so the SPMD program is uniform across cores; its
result is discarded and identity rows are copied exactly on the host).
Token data is packed transposed ([D, C], feature on partition axis) so the
device does pure dense streaming matmuls - no gather/transpose on device.

Per-core work: ~(B*L/8) tokens * 2*D^2 flops (fp32r ~= full PE rate) and
~4MB weights + 2*C*4KB activations of HBM traffic - right at the
compute/memory ridge.
"""

import sys

for _p in ("/root/.axon_site/_ro/trn_rl_repo", "/opt/trn_rl_repo"):
    if _p not in sys.path:
        sys.path.append(_p)

import numpy as np
import concourse.bass as bass
import concourse.tile as tile
from concourse import bacc, mybir
from concourse.bass_utils import run_bass_kernel_spmd

D = 1024
P = 128
N_CORES = 8
TT = 512  # token tile (moving-dim block)
F32 = mybir.dt.float32
F32R = mybir.dt.float32r

_program_cache: dict[int, bass.Bass] = {}


def build_program(C: int) -> bass.Bass:
    """out[C, D] = xT.T @ wT + bB   (xT: [D, C], wT: [D, D] both [contract, free])."""
    if C in _program_cache:
        return _program_cache[C]
    nc = bacc.Bacc("TRN2", target_bir_lowering=False, debug=False, num_devices=N_CORES)
    # Inputs declared fp32r so matmul operands come straight from DMA with no
    # on-device conversion pass (fp32r rounding happens in the PE datapath).
    xT = nc.dram_tensor("xT", [D, C], F32R, kind="ExternalInput")
    wT = nc.dram_tensor("wT", [D, D], F32R, kind="ExternalInput")
    bB = nc.dram_tensor("bB", [P, D], F32, kind="ExternalInput")
    out = nc.dram_tensor("out", [C, D], F32, kind="ExternalOutput")

    n_ic = D // P  # contraction chunks
    n_ob = D // TT  # output blocks

    # Uniform 512-token tiles (last = remainder).
    t_tiles = []
    t0 = 0
    while t0 < C:
        tt = min(TT, C - t0)
        t_tiles.append((t0, tt))
        t0 += tt

    with tile.TileContext(nc) as tc:
        with (
            tc.tile_pool(name="wpool", bufs=1) as wpool,
            tc.tile_pool(name="bpool", bufs=1) as bpool,
            tc.tile_pool(name="xpool", bufs=4) as xpool,
            tc.tile_pool(name="opool", bufs=8) as opool,
            tc.tile_pool(name="psum", bufs=1, space="PSUM") as psum_pool,
        ):
            # Interleave x(t0)[ic] with w[ic] in ic order: the ic-outer matmul
            # schedule consumes exactly (x[ic], w[ic]) per stage, so the PE
            # starts after ~1MB of DMA and self-paces against the stream.
            first_tt = t_tiles[0][1]
            x_first = []
            w_tiles = []
            b_tile = None
            for ic in range(n_ic):
                xt = xpool.tile([P, first_tt], F32R, name=f"x_first{ic}", tag=f"x{ic}")
                nc.sync.dma_start(xt[:], xT[ic * P : (ic + 1) * P, :first_tt])
                x_first.append(xt)
                wt = wpool.tile([P, D], F32R, name=f"w{ic}", tag=f"w{ic}")
                nc.sync.dma_start(wt[:], wT[ic * P : (ic + 1) * P, :])
                w_tiles.append(wt)
                if ic == 2:
                    b_tile = bpool.tile([P, D], F32, name="b_tile")
                    nc.sync.dma_start(b_tile[:], bB[:])

            for ti, (t0, tt) in enumerate(t_tiles):
                if ti == 0:
                    x_tiles = x_first
                else:
                    x_tiles = []
                    for ic in range(n_ic):
                        xt = xpool.tile([P, tt], F32R, tag=f"x{ic}")
                        nc.sync.dma_start(
                            xt[:], xT[ic * P : (ic + 1) * P, t0 : t0 + tt]
                        )
                        x_tiles.append(xt)
                # One psum bank per (token-chunk, output-block) group; run the
                # contraction ic-outer across all banks so each weight chunk
                # is needed only once per ~8 matmuls (JIT weight streaming).
                groups = [
                    (tc_, ob) for tc_ in range(tt // P) for ob in range(n_ob)
                ]
                ps = {
                    g: psum_pool.tile(
                        [P, TT], F32, name=f"ps_{ti}_{gi}", tag=f"ps{gi % 8}"
                    )
                    for gi, g in enumerate(groups)
                }
                for ic in range(n_ic):
                    for tc_, ob in groups:
                        nc.tensor.matmul(
                            ps[(tc_, ob)][:],
                            x_tiles[ic][:, tc_ * P : (tc_ + 1) * P],
                            w_tiles[ic][:, ob * TT : (ob + 1) * TT],
                            start=(ic == 0),
                            stop=(ic == n_ic - 1),
                        )
                for tc_, ob in groups:
                    ot = opool.tile([P, TT], F32)
                    nc.vector.tensor_add(
                        ot[:], ps[(tc_, ob)][:], b_tile[:, ob * TT : (ob + 1) * TT]
                    )
                    nc.gpsimd.dma_start(
                        out[
                            t0 + tc_ * P : t0 + (tc_ + 1) * P,
                            ob * TT : (ob + 1) * TT,
                        ],
                        ot[:],
                    )
    nc.compile()
    _program_cache[C] = nc
    return nc


def kernel(actions, action_type, W, b, _trace=False):
    actions = np.ascontiguousarray(actions, dtype=np.float32)
    B, L, _ = actions.shape
    flat = actions.reshape(B * L, D)
    types = np.asarray(action_type).reshape(B * L).astype(np.int64)

    idx = [np.flatnonzero(types == t) for t in range(N_CORES)]
    counts = [len(i) for i in idx]
    C = max(P, -(-max(counts) // P) * P)  # pad to multiple of 128

    W = np.asarray(W, dtype=np.float32)
    b_np = np.asarray(b, dtype=np.float32)
    eye = np.eye(D, dtype=np.float32)

    in_maps = []
    for t in range(N_CORES):
        xT = np.zeros((D, C), dtype=np.float32)
        if t > 0 and counts[t]:
            xT[:, : counts[t]] = flat[idx[t]].T
        wT = eye if t == 0 else np.ascontiguousarray(W[t - 1].T)
        bvec = np.zeros(D, dtype=np.float32) if t == 0 else b_np[t - 1]
        bB = np.ascontiguousarray(np.broadcast_to(bvec, (P, D)))
        in_maps.append({"xT": xT, "wT": wT, "bB": bB})

    nc = build_program(C)
    r = run_bass_kernel_spmd(nc, in_maps, list(range(N_CORES)), trace=_trace)

    out_flat = np.empty_like(flat)
    out_flat[idx[0]] = flat[idx[0]]  # identity tokens: exact copy
    for t in range(1, N_CORES):
        if counts[t]:
            out_flat[idx[t]] = r.results[t]["out"][: counts[t]]
    out = out_flat.reshape(B, L, D)
    if _trace:
        return out, r
    return out
